# revision 6
# baseline (speedup 1.0000x reference)
"""Trainium2 Bass kernel for: 1x1-conv GEMM + GroupNorm + HardTanh.

Reference computation (per sample b):
    y = weight @ x[b]                        # [512, 256] @ [256, 56*56]
    groupnorm over 32 groups of 16 channels  # stats over (16, 56*56)
    y = y * gamma + beta                     # per-channel affine
    out = clip(y, -2, 2)

Sharding: data-parallel over batch, 4 samples per core x 8 cores.
weight/gamma/beta replicated. No cross-core communication needed.

HBM-bandwidth-bound by design: x and the output travel as fp16
(halving DMA traffic vs fp32; fp16's 10 mantissa bits keep the
end-to-end error ~1e-3 of scale) and the matmul runs in fp16 at the
full PE rate. Each 128-channel chunk is processed B-half (cols
2048:3136) FIRST, then A-half (0:2048), so the DVE's immediate
square+reduce work lands right behind the small copy while the big
A-half copy streams in parallel. Engine assignment:
  PE   : 512-col matmuls into a 3-bank B tile + 4-bank A tile (one
         rotating slot pair, chunk k+1 fills while k drains), plus a
         tiny per-sample group-aggregation matmul.
  ACT  : sole PSUM reader - Copy PSUM fp32 -> SBUF fp16 with accum_out
         giving per-channel sum(y) as a free side effect.
  Pool : squares the A-half head columns (TensorTensor mult fp16).
         The matching sum-reduce runs one chunk LATE on the DVE so
         Pool latency stays entirely off the stats critical path.
  DVE  : squares the B-half + A-tail (TT 2x mode); in 4x mode the
         region sum(y^2) reduces, the affine (mult s, add bv) and the
         clamp (min 2, max -2).
Group stats are per-SAMPLE: 16 per-chunk accumulators feed one tiny PE
matmul against a replicated block-diagonal averaging matrix
(group-reduce + broadcast in one shot) then a short DVE/ACT chain.
Sample b's affine/clamp/store are software-pipelined into sample b+1's
chunk loop so the in-order DVE stream never waits on the stats chain
and output DMAs spread evenly; the trailing sample splits its clamps
DVE/Pool to compress the drain.
"""

import sys

sys.path.insert(0, "/opt/trn_rl_repo")

import numpy as np

import concourse.bacc as bacc
import concourse.mybir as mybir
import concourse.tile as tile
from concourse.bass_utils import run_bass_kernel_spmd

# Problem shape (hardcoded per contest contract)
B, CIN, COUT, H, W = 32, 256, 512, 56, 56
HW = H * W  # 3136
G = 32  # num groups
GSIZE = COUT // G  # 16 channels per group
EPS = 1e-5
HT_MIN, HT_MAX = -2.0, 2.0

N_CORES = 8
BPC = B // N_CORES  # samples per core = 4
KC = CIN // 128  # contraction chunks = 2
OC = COUT // 128  # output-channel chunks = 4

HWA = 2048  # A half: hw cols [0, 2048)
HWB = HW - HWA  # B half: [2048, 3136) = 1088 cols
A_TILES = [(t * 512, 512) for t in range(4)]
B_TILES = [(0, 512), (512, 512), (1024, 64)]
GPS_OFF = 1536  # group-stats scratch cols inside the last B psum tile

# gpsimd square-pass head sizes per chunk (within the A half); small on
# the last chunk so the sample's final A reduce doesn't wait on Pool
TP = [1740, 1740, 1740, 400]
# drain-phase clamp split (trailing sample only)
DRAIN_POOL = 1024

_NC_CACHE = None


def _build_program():
    f32 = mybir.dt.float32
    f16 = mybir.dt.float16

    nc = bacc.Bacc("TRN2", target_bir_lowering=False, debug=False)

    x_d = nc.dram_tensor("x", [BPC, CIN, HW], f16, kind="ExternalInput")
    wt_d = nc.dram_tensor("wt", [CIN, COUT], f16, kind="ExternalInput")
    gamma_d = nc.dram_tensor("gamma", [COUT], f32, kind="ExternalInput")
    beta_d = nc.dram_tensor("beta", [COUT], f32, kind="ExternalInput")
    agg_d = nc.dram_tensor("agg", [128, 128], f32, kind="ExternalInput")
    out_d = nc.dram_tensor("out", [BPC, COUT, HW], f16, kind="ExternalOutput")

    with tile.TileContext(nc) as tc:
        with (
            tc.tile_pool(name="singles", bufs=1) as singles,
            tc.tile_pool(name="xp", bufs=2) as xp,
            tc.tile_pool(name="yp", bufs=8) as yp,
            tc.tile_pool(name="up", bufs=3) as up,
            tc.tile_pool(name="fp", bufs=4) as fp,
            tc.tile_pool(name="tp", bufs=3) as tp,
            tc.tile_pool(name="small", bufs=2) as small,
            tc.tile_pool(name="psy", bufs=2, space="PSUM") as psy,
        ):
            # --- one-time setup -------------------------------------------
            # the B-half sliver of sample 0 goes first (the first matmuls
            # read it), then weights, then the rest of x; scalars on SWDGE
            x0_sb = xp.tile([128, KC, HW], f16, tag="x")

            def load_x_part(x_tile, b, lo, hi):
                nc.sync.dma_start(
                    out=x_tile[:, :, lo:hi],
                    in_=x_d.ap()[b, :, lo:hi].rearrange(
                        "(c p) f -> p c f", p=128
                    ),
                )

            load_x_part(x0_sb, 0, HWA, HW)
            wt_sb = singles.tile([128, KC, COUT], f16)
            nc.sync.dma_start(
                out=wt_sb, in_=wt_d.ap().rearrange("(c p) m -> p c m", p=128)
            )
            load_x_part(x0_sb, 0, 0, 1024)
            load_x_part(x0_sb, 0, 1024, HWA)
            gamma_sb = singles.tile([128, OC], f32)
            nc.gpsimd.dma_start(
                out=gamma_sb, in_=gamma_d.ap().rearrange("(c p) -> p c", p=128)
            )
            beta_sb = singles.tile([128, OC], f32)
            nc.gpsimd.dma_start(
                out=beta_sb, in_=beta_d.ap().rearrange("(c p) -> p c", p=128)
            )
            eps_sb = singles.tile([128, 1], f32)
            nc.vector.memset(eps_sb, EPS)
            agg_sb = singles.tile([128, 128], f32)
            nc.gpsimd.dma_start(out=agg_sb, in_=agg_d.ap())

            x_tiles = [x0_sb]

            # pipeline state
            y_tiles = {}  # (b, oc) -> y_sb
            y2_tiles = {}  # (b, oc) -> y2
            sums_t = {}  # b -> accumulator tile
            sb_t = {}  # b -> (s4, bv4)

            def mm_half(ps, tiles, base, x_sb, osl):
                for lo, wdt in tiles:
                    for c in range(KC):
                        nc.tensor.matmul(
                            ps[:, lo : lo + wdt],
                            wt_sb[:, c, osl],
                            x_sb[:, c, base + lo : base + lo + wdt],
                            start=(c == 0),
                            stop=(c == KC - 1),
                        )

            def emit_chunk(b, oc):
                """matmuls + PSUM evacuation + squares + B reduce.

                B half first: its copy and square/reduce are the only
                stats work on the immediate critical path. The A-half
                reduce is emitted one chunk later (sum_a)."""
                x_sb = x_tiles[b]
                osl = slice(oc * 128, (oc + 1) * 128)
                sums = sums_t[b]

                psB = psy.tile([128, 2048], f32, tag="ps")
                psA = psy.tile([128, 2048], f32, tag="ps")
                y_sb = yp.tile([128, HW], f16, tag="y")
                y_tiles[(b, oc)] = y_sb
                y2 = tp.tile([128, HW], f16, tag="y2")
                y2_tiles[(b, oc)] = y2
                tp_c = TP[oc]

                # first chunk of a sample runs A before B so the psB slot
                # (which carries the previous sample's group stats in its
                # tail) has time to finish its chain reads
                if oc == 0:
                    mm_half(psA, A_TILES, 0, x_sb, osl)
                    mm_half(psB, B_TILES, HWA, x_sb, osl)
                else:
                    mm_half(psB, B_TILES, HWA, x_sb, osl)
                    mm_half(psA, A_TILES, 0, x_sb, osl)

                nc.scalar.activation(
                    out=y_sb[:, HWA:HW],
                    in_=psB[:, 0:HWB],
                    func=mybir.ActivationFunctionType.Copy,
                    accum_out=sums[:, OC + oc : OC + oc + 1],
                )
                nc.scalar.activation(
                    out=y_sb[:, 0:HWA],
                    in_=psA[:, 0:HWA],
                    func=mybir.ActivationFunctionType.Copy,
                    accum_out=sums[:, oc : oc + 1],
                )

                # squares: DVE takes the B half now, Pool the A-half head
                # (only needs the A copy); the A tail is squared by DVE at
                # the end of this chunk's stream where it can't stall
                nc.vector.tensor_mul(
                    y2[:, HWA:HW], y_sb[:, HWA:HW], y_sb[:, HWA:HW]
                )
                trash = tp.tile([128, HW], f16, tag="t")
                nc.vector.tensor_scalar(
                    out=trash[:, HWA:HW],
                    in0=y2[:, HWA:HW],
                    scalar1=1.0,
                    scalar2=None,
                    op0=mybir.AluOpType.mult,
                    op1=mybir.AluOpType.add,
                    accum_out=sums[:, 3 * OC + oc : 3 * OC + oc + 1],
                )
                nc.gpsimd.tensor_mul(
                    y2[:, 0:tp_c], y_sb[:, 0:tp_c], y_sb[:, 0:tp_c]
                )
                return psB

            def square_a_tail(b, oc):
                tp_c = TP[oc]
                y_sb, y2 = y_tiles[(b, oc)], y2_tiles[(b, oc)]
                nc.vector.tensor_mul(
                    y2[:, tp_c:HWA], y_sb[:, tp_c:HWA], y_sb[:, tp_c:HWA]
                )

            def sum_a(b, oc):
                """A-half sum(y^2): one chunk behind its squares."""
                sums = sums_t[b]
                trash = tp.tile([128, HW], f16, tag="t")
                nc.vector.tensor_scalar(
                    out=trash[:, 0:HWA],
                    in0=y2_tiles[(b, oc)][:, 0:HWA],
                    scalar1=1.0,
                    scalar2=None,
                    op0=mybir.AluOpType.mult,
                    op1=mybir.AluOpType.add,
                    accum_out=sums[:, 2 * OC + oc : 2 * OC + oc + 1],
                )

            def emit_chain(b, ps_b3):
                """per-sample group stats -> per-channel scale/bias."""
                sums = sums_t[b]
                gps = ps_b3[:, GPS_OFF : GPS_OFF + 4 * OC]
                nc.tensor.matmul(
                    gps, agg_sb, sums, start=True, stop=True,
                    skip_group_check=True,
                )
                gs = small.tile([128, 4 * OC], f32, tag="gs")
                nc.vector.tensor_copy(out=gs, in_=gps)
                m4 = small.tile([128, OC], f32, tag="m4")
                nc.vector.tensor_add(m4, gs[:, 0:OC], gs[:, OC : 2 * OC])
                q4 = small.tile([128, OC], f32, tag="q4")
                nc.vector.tensor_add(
                    q4, gs[:, 2 * OC : 3 * OC], gs[:, 3 * OC : 4 * OC]
                )
                msq = small.tile([128, OC], f32, tag="msq")
                nc.vector.tensor_mul(msq, m4, m4)
                ve = small.tile([128, OC], f32, tag="ve")
                nc.vector.tensor_sub(ve, q4, msq)
                sd = small.tile([128, OC], f32, tag="sd")
                nc.scalar.activation(
                    out=sd,
                    in_=ve,
                    func=mybir.ActivationFunctionType.Sqrt,
                    bias=eps_sb,
                )
                rstd = small.tile([128, OC], f32, tag="rstd")
                nc.vector.reciprocal(rstd, sd)
                s4 = small.tile([128, OC], f32, tag="s4")
                nc.vector.tensor_mul(s4, rstd, gamma_sb)
                ms = small.tile([128, OC], f32, tag="ms")
                nc.vector.tensor_mul(ms, m4, s4)
                bv4 = small.tile([128, OC], f32, tag="bv4")
                nc.vector.tensor_sub(bv4, beta_sb, ms)
                sb_t[b] = (s4, bv4)

            def emit_transform(b, oc, drain=False):
                """affine + clamp + store. In the drain phase the clamp
                splits DVE/Pool (Pool is idle there) with two stores."""
                s4, bv4 = sb_t[b]
                osl = slice(oc * 128, (oc + 1) * 128)
                u_sb = up.tile([128, HW], f16, tag="u")
                nc.vector.tensor_scalar(
                    out=u_sb,
                    in0=y_tiles.pop((b, oc)),
                    scalar1=s4[:, oc : oc + 1],
                    scalar2=bv4[:, oc : oc + 1],
                    op0=mybir.AluOpType.mult,
                    op1=mybir.AluOpType.add,
                )
                f_sb = fp.tile([128, HW], f16, tag="f")
                if drain:
                    nc.gpsimd.tensor_scalar(
                        out=f_sb[:, 0:DRAIN_POOL],
                        in0=u_sb[:, 0:DRAIN_POOL],
                        scalar1=HT_MAX,
                        scalar2=HT_MIN,
                        op0=mybir.AluOpType.min,
                        op1=mybir.AluOpType.max,
                    )
                    nc.vector.tensor_scalar(
                        out=f_sb[:, DRAIN_POOL:HW],
                        in0=u_sb[:, DRAIN_POOL:HW],
                        scalar1=HT_MAX,
                        scalar2=HT_MIN,
                        op0=mybir.AluOpType.min,
                        op1=mybir.AluOpType.max,
                    )
                    nc.sync.dma_start(
                        out=out_d.ap()[b, osl, DRAIN_POOL:HW],
                        in_=f_sb[:, DRAIN_POOL:HW],
                    )
                    nc.sync.dma_start(
                        out=out_d.ap()[b, osl, 0:DRAIN_POOL],
                        in_=f_sb[:, 0:DRAIN_POOL],
                    )
                else:
                    nc.vector.tensor_scalar(
                        out=f_sb,
                        in0=u_sb,
                        scalar1=HT_MAX,
                        scalar2=HT_MIN,
                        op0=mybir.AluOpType.min,
                        op1=mybir.AluOpType.max,
                    )
                    nc.sync.dma_start(out=out_d.ap()[b, osl, :], in_=f_sb)

            # --- main software-pipelined loop -----------------------------
            for b in range(BPC):
                sums_t[b] = small.tile(
                    [128, 4 * OC], f32, tag="sums", name="sums"
                )
                ps_b3 = None
                for oc in range(OC):
                    if b + 1 < BPC and oc < 2:
                        if oc == 0:
                            xnext = xp.tile([128, KC, HW], f16, tag="x")
                            x_tiles.append(xnext)
                            load_x_part(x_tiles[b + 1], b + 1, HWA, HW)
                            load_x_part(x_tiles[b + 1], b + 1, 0, 1024)
                        else:
                            load_x_part(x_tiles[b + 1], b + 1, 1024, HWA)
                    ps_b3 = emit_chunk(b, oc)
                    if oc > 0:
                        sum_a(b, oc - 1)
                    # previous sample's transform slots in here, keeping
                    # the DVE stream busy while this sample's stats build
                    if b > 0:
                        emit_transform(b - 1, oc)
                    square_a_tail(b, oc)
                sum_a(b, OC - 1)
                emit_chain(b, ps_b3)
            for oc in range(OC):
                emit_transform(BPC - 1, oc, drain=True)

    nc.compile()
    return nc


def _get_program():
    global _NC_CACHE
    if _NC_CACHE is None:
        _NC_CACHE = _build_program()
    return _NC_CACHE


def _make_in_maps(x, weight, gamma, beta):
    xr = np.ascontiguousarray(x.reshape(B, CIN, HW).astype(np.float16))
    wt = np.ascontiguousarray(weight.T.astype(np.float16))  # [CIN, COUT]
    gamma = np.ascontiguousarray(gamma, dtype=np.float32)
    beta = np.ascontiguousarray(beta, dtype=np.float32)
    agg = np.zeros((128, 128), dtype=np.float32)
    inv = 1.0 / (GSIZE * HW)
    for g in range(128 // GSIZE):
        agg[g * GSIZE : (g + 1) * GSIZE, g * GSIZE : (g + 1) * GSIZE] = inv
    return [
        {
            "x": xr[i * BPC : (i + 1) * BPC],
            "wt": wt,
            "gamma": gamma,
            "beta": beta,
            "agg": agg,
        }
        for i in range(N_CORES)
    ]


def kernel(x, weight, gamma, beta):
    x = np.asarray(x, dtype=np.float32)
    weight = np.asarray(weight, dtype=np.float32)
    assert x.shape == (B, CIN, H, W)
    nc = _get_program()
    in_maps = _make_in_maps(x, weight, gamma, beta)
    res = run_bass_kernel_spmd(nc, in_maps, core_ids=list(range(N_CORES)))
    out = np.concatenate([r["out"] for r in res.results], axis=0)
    return out.astype(np.float32).reshape(B, COUT, H, W)
